# revision 13
# baseline (speedup 1.0000x reference)
"""ChebConv (K=5) Trainium2 kernel — 8-core SPMD, bf16 state table.

Strategy (row-sharded, all-batches-per-row):
  - State table X_k [M=32768, 256] bf16 in HBM: row m holds all 8 samples x 32 feats
    (512B rows). Each core owns a 4096-row quarter: computes Y = L @ X for its rows.
  - Gather: dma_gather (SWDGE), 512B descriptors, edge-major SBUF chunks [128 slots, 256];
    deep ring + sparse pacing waits keep the Q7 descriptor stream busy.
  - Segment-sum + vals: PE matmuls psum[64 rows, 256] += W_chunk^T @ g_chunk, where
    W_chunk [128 slots, 64 rows] carries vals at (slot, row-row0). W is built ON DEVICE
    (one DVE op per chunk: (iota==rr)*val from compact per-slot rr/val arrays) and lives
    entirely in SBUF (bf16, 40KB/partition) — no HBM W stream.
  - Chebyshev: T_k = 2 (L T_{k-1}) - T_{k-2} on DVE (psum f32 -> bf16 strips); T_k strips
    written to HBM, exchanged across cores via AllGather -> next step's gather table.
  - Output: out += T_k^T @ kernel_k per step (PE bf16 transpose + small matmuls, f32 acc),
    final copy to bf16 for the (halved) device->host fetch.
Host does data layout: ELL windows (64 rows -> 640 slots = 5 chunks of 128), compact
int16 idx + f32 val/row-offset slot streams, strip/kernel/bias layouts. All staged
inputs are compact/bf16 (~2.6MB/core vs 15.5MB f32-dense) since host->device staging
through the axon tunnel dominates end-to-end wall time.

_build_nc(repeat=R) unrolls the whole pipeline R times (same data, same output) so
device execution time can be measured as (wall(R) - wall(1)) / (R - 1), independent
of host staging/dispatch overhead.
"""
import os
import sys

sys.path.insert(0, "/opt/trn_rl_repo")

import numpy as np
import ml_dtypes

BF16 = ml_dtypes.bfloat16

NB, M, FIN = 8, 32768, 32
RANK, FILT = 5, 32
E = 262144
NCORES = 8
QROWS = M // NCORES           # 4096 rows per core
WROWS = 64                    # rows per window (pair)
NWIN = QROWS // WROWS         # 64 windows per core
WSLOTS = 640                  # slots per window (5 chunks x 128)
NCHUNK_W = WSLOTS // 128      # 5
NSLOT = NWIN * WSLOTS         # 40960 slots per core
NCHUNK = NSLOT // 128         # 320 chunks per step
F256 = NB * FIN               # 256

# gather pipeline config
G_IDX = 1024                  # idxs per dma_gather call
NQ = 1                        # SWDGE queues used
GRING = 8                     # g_sb ring depth (groups in flight)
PACE = 4                      # pacing wait every PACE gather calls
NCALL_G = NSLOT // G_IDX      # gather calls per step
CW = G_IDX // 128             # chunks per gather group

_cache = {}


def _build_host_data(vals, kern, bias, rows, cols):
    rows = np.asarray(rows); cols = np.asarray(cols); vals = np.asarray(vals)
    idx_all = []
    wv_all = []
    wr_all = []
    order = np.argsort(rows, kind="stable")
    rs, cs, vs = rows[order], cols[order], vals[order]
    starts = np.searchsorted(rs, np.arange(0, M + 1, WROWS))
    for c in range(NCORES):
        idx_stream = np.zeros(NSLOT, dtype=np.int16)
        v_slot = np.zeros(NSLOT, dtype=np.float32)
        r_slot = np.zeros(NSLOT, dtype=np.float32)
        for w in range(NWIN):
            gw = c * NWIN + w
            lo, hi = starts[gw], starts[gw + 1]
            n = hi - lo
            assert n <= WSLOTS, f"window overflow: {n} > {WSLOTS}"
            base = w * WSLOTS
            idx_stream[base:base + n] = cs[lo:hi].astype(np.int16)
            v_slot[base:base + n] = vs[lo:hi]
            r_slot[base:base + n] = (rs[lo:hi] - gw * WROWS).astype(np.float32)
        # gather idx wrap: idx_sb[p, t] = idx_stream[16t + p] (16 partitions,
        # replicated x8 on device)
        idx_all.append(np.ascontiguousarray(idx_stream.reshape(NSLOT // 16, 16).T))
        # per-slot (chunk, slot-in-chunk) layout: [128 st, NCHUNK]
        wv_all.append(np.ascontiguousarray(v_slot.reshape(NCHUNK, 128).T))
        wr_all.append(np.ascontiguousarray(r_slot.reshape(NCHUNK, 128).T))
    krep = np.zeros((128, RANK, 128), dtype=np.float32)
    for k in range(RANK):
        for q in range(4):
            for f in range(FIN):
                krep[32 * q + f, k, 32 * q:32 * (q + 1)] = kern[f * RANK + k, :]
    krep = krep.astype(BF16)
    bias_rep = np.tile(bias.astype(np.float32)[None, :], (128, NB)).reshape(128, F256).astype(BF16)
    ident = np.tile(np.eye(64, dtype=np.float32), (2, 1)).astype(BF16)
    iota = np.tile(np.arange(64, dtype=np.float32)[None, :], (128, 1))
    return idx_all, wv_all, wr_all, krep, bias_rep, ident, iota


def _strip_layout(x_table_bf, c):
    """core c's 4096 bf16 rows -> strip tensor [128, 32, 256]."""
    a = x_table_bf[c * QROWS:(c + 1) * QROWS].reshape(NWIN // 2, 2, WROWS, F256)
    return np.ascontiguousarray(a.transpose(1, 2, 0, 3).reshape(128, NWIN // 2, F256))


def _build_nc(repeat=1):
    from concourse import bass, bacc, mybir
    from concourse.library_config import mlp

    ABL = os.environ.get("ABL", "")
    no_acc = ABL == "no_acc"      # drop output-accumulation phases
    no_ag = ABL == "no_ag"        # drop AllGathers (gathers read stale ag0)
    no_cheb = ABL == "no_cheb"    # DVE cheb -> nop (keeps sync structure)
    cheb_act = ABL == "cheb_act"  # cheb psum drain on ACT engine (test numerics wrong)
    cheb_sbuf = ABL == "cheb_sbuf"  # cheb stt reads SBUF garbage instead of psum

    f32 = mybir.dt.float32
    bf16 = mybir.dt.bfloat16
    nc = bacc.Bacc("TRN2", target_bir_lowering=False, debug=False,
                   num_devices=NCORES, num_swdge_queues=NQ)
    xs_d = nc.dram_tensor("xs", [128, NWIN // 2, F256], bf16, kind="ExternalInput")
    idx_d = nc.dram_tensor("idx", [16, NSLOT // 16], mybir.dt.int16, kind="ExternalInput")
    wv_d = nc.dram_tensor("wv", [128, NCHUNK], f32, kind="ExternalInput")
    wr_d = nc.dram_tensor("wr", [128, NCHUNK], f32, kind="ExternalInput")
    krep_d = nc.dram_tensor("krep", [128, RANK, 128], bf16, kind="ExternalInput")
    bias_d = nc.dram_tensor("biasr", [128, F256], bf16, kind="ExternalInput")
    id_d = nc.dram_tensor("ident", [128, 64], bf16, kind="ExternalInput")
    iota_d = nc.dram_tensor("iotar", [128, 64], f32, kind="ExternalInput")
    out_d = nc.dram_tensor("out", [128, NWIN // 2, F256], bf16, kind="ExternalOutput")

    wb_l = [nc.dram_tensor(f"wb{k}", [QROWS, F256], bf16) for k in (0, 1, 2, 3)]
    ag_l = [nc.dram_tensor(f"ag{k}", [M, F256], bf16, addr_space="Shared") for k in (0, 1, 2, 3)]

    NSTEP = 4 * repeat            # global SpMM steps
    STRIPS = 5 * NWIN             # acc strips per iteration (320)

    from contextlib import ExitStack
    with ExitStack() as _stk:
        block = _stk.enter_context(nc.Block())
        idx_sb = _stk.enter_context(nc.sbuf_tensor("idx_sb", [128, NSLOT // 16], mybir.dt.int16))
        g_sb = _stk.enter_context(nc.sbuf_tensor("g_sb", [128, GRING, CW, F256], bf16))
        wv_sb = _stk.enter_context(nc.sbuf_tensor("wv_sb", [128, NCHUNK], f32))
        wr_sb = _stk.enter_context(nc.sbuf_tensor("wr_sb", [128, NCHUNK], f32))
        w_all = _stk.enter_context(nc.sbuf_tensor("w_all", [128, NCHUNK, WROWS], bf16))
        iota_sb = _stk.enter_context(nc.sbuf_tensor("iota_sb", [128, 64], f32))
        ql = _stk.enter_context(nc.sbuf_tensor("ql", [128, 2, NWIN // 2, F256], bf16))
        acc = _stk.enter_context(nc.sbuf_tensor("acc", [128, NWIN // 2, F256], f32))
        obf = _stk.enter_context(nc.sbuf_tensor("obf", [128, NWIN // 2, F256], bf16))
        fm = _stk.enter_context(nc.sbuf_tensor("fm", [128, 2, 2, WROWS], bf16))
        krep_sb = _stk.enter_context(nc.sbuf_tensor("krep_sb", [128, RANK, 128], bf16))
        bias_sb = _stk.enter_context(nc.sbuf_tensor("bias_sb", [128, F256], bf16))
        id_sb = _stk.enter_context(nc.sbuf_tensor("id_sb", [128, 64], bf16))
        io = _stk.enter_context(nc.semaphore("io"))
        gsem = [_stk.enter_context(nc.semaphore(f"gsem{i}")) for i in range(NQ)]
        segd = _stk.enter_context(nc.semaphore("segd"))
        psfree = _stk.enter_context(nc.semaphore("psfree"))
        chebd = _stk.enter_context(nc.semaphore("chebd"))
        tpd = _stk.enter_context(nc.semaphore("tpd"))
        fmcp = _stk.enter_context(nc.semaphore("fmcp"))
        accmm = _stk.enter_context(nc.semaphore("accmm"))
        accfree = _stk.enter_context(nc.semaphore("accfree"))
        wbd = _stk.enter_context(nc.semaphore("wbd"))
        xsl = _stk.enter_context(nc.semaphore("xsl"))
        wbs = [_stk.enter_context(nc.semaphore(f"wbs{i}")) for i in range(4)]
        ccs = _stk.enter_context(nc.semaphore("ccs"))
        outs = _stk.enter_context(nc.semaphore("outs"))
        psum_seg = [nc.alloc_psum_tensor(f"ps{i}", [64, 512], f32) for i in range(2)]
        psum_tp = [nc.alloc_psum_tensor(f"pt{i}", [128, 1024], bf16) for i in range(2)]
        psum_acc = [nc.alloc_psum_tensor(f"pa{i}", [64, 512], f32) for i in range(2)]

        NPRO = 15  # prologue input DMAs (xs, wv, wr, krep, biasr, ident, iotar + 8 idx)

        def ql_strip(gen, s):
            return ql[(s % 2) * 64:(s % 2) * 64 + 64, gen % 2, s // 2, :]

        def acc_strip(s):
            return acc[(s % 2) * 64:(s % 2) * 64 + 64, s // 2, :]

        def consumed(J_g):
            """segd count proving PE consumed all chunks of gather group J_g."""
            S_of, j = divmod(J_g, NCALL_G)
            last_ch = CW * (j + 1) - 1
            return S_of * NWIN + last_ch // NCHUNK_W + 1

        # ---------------- GPSIMD: AllGathers + gathers ----------------
        @block.gpsimd
        def _(gp: bass.BassGpSimd):
            gp.load_library(mlp)
            gp.wait_ge(io, 16 * NPRO)
            gp.wait_ge(wbs[0], 16)
            if not no_ag:
                gp.collective_compute(
                    "AllGather", bass.mybir.AluOpType.bypass,
                    replica_groups=[list(range(NCORES))],
                    ins=[wb_l[0][:]], outs=[ag_l[0][:]],
                ).then_inc(ccs, 1)
            for S in range(NSTEP):
                k = 0 if no_ag else S % 4
                if not no_ag:
                    gp.wait_ge(ccs, S + 1)
                for j in range(NCALL_G):
                    J_g = S * NCALL_G + j
                    if J_g >= GRING and J_g % PACE == 0:
                        gp.wait_ge(segd, consumed(J_g - GRING + PACE - 1))
                    gp.dma_gather(
                        g_sb[:, J_g % GRING, :, :], ag_l[k][:],
                        idx_sb[:, (G_IDX // 16) * j:(G_IDX // 16) * (j + 1)],
                        G_IDX, G_IDX, F256,
                        queue_num=J_g % NQ,
                    ).then_inc(gsem[J_g % NQ], 16)
                if S + 1 < NSTEP and not no_ag:
                    kn = (S + 1) % 4
                    itn = (S + 1) // 4
                    gp.wait_ge(wbs[kn], 16 * (itn + 1) if kn > 0 else 16)
                    gp.collective_compute(
                        "AllGather", bass.mybir.AluOpType.bypass,
                        replica_groups=[list(range(NCORES))],
                        ins=[wb_l[kn][:]], outs=[ag_l[kn][:]],
                    ).then_inc(ccs, 1)

        # ---------------- PE ----------------
        @block.tensor
        def _(pe: bass.BassTensorEngine):
            pe.wait_ge(io, 16 * NPRO)

            def acc_phase(it, ka):
                if no_acc:
                    return
                # strips of T_ka (iteration it) -> transposes + acc mms
                for s in range(NWIN):
                    ST = it * STRIPS + ka * NWIN + s
                    if ka > 0:
                        pe.wait_ge(chebd, it * 4 * NWIN + (ka - 1) * NWIN + s + 1)
                    for h in range(2):
                        t = 2 * ST + h
                        if t >= 2:
                            pe.wait_ge(fmcp, t - 1)  # tp psum ring free
                        sb = (s % 2) * 64
                        pe.transpose(
                            out=psum_tp[t % 2][:, :WROWS],
                            in_=ql_strip(ka, s)[:, 128 * h:128 * (h + 1)],
                            identity=id_sb[sb:sb + 64, :],
                        ).then_inc(tpd, 1)
                    if ST >= 2:
                        pe.wait_ge(accfree, ST - 1)
                    pe.wait_ge(fmcp, 2 * ST + 2)
                    for h in range(2):
                        mmacc = pe.matmul(
                            out=psum_acc[ST % 2][:, 128 * h:128 * (h + 1)],
                            lhsT=fm[:, ST % 2, h, :],
                            rhs=krep_sb[:, ka, :],
                            start=True, stop=True,
                        )
                        if h == 1:
                            mmacc.then_inc(accmm, 1)

            for it in range(repeat):
                if it >= 1:
                    pe.wait_ge(xsl, 16 * it)  # xs reloaded into ql gen0
                acc_phase(it, 0)
                if it == 0:
                    pe.wait_ge(wbd, 1)  # on-device W build complete
                for k in range(1, RANK):
                    S = it * 4 + k - 1
                    for p in range(NWIN):
                        P = S * NWIN + p
                        if P >= 2:
                            pe.wait_ge(psfree, P - 1)
                        for i in range(NCHUNK_W):
                            ch = NCHUNK_W * p + i
                            J_g = S * NCALL_G + ch // CW
                            pe.wait_ge(gsem[J_g % NQ], 16 * (J_g // NQ + 1))
                            mm = pe.matmul(
                                out=psum_seg[P % 2][:, :F256],
                                lhsT=w_all[:, ch, :],
                                rhs=g_sb[:, J_g % GRING, (ch % CW), :],
                                start=(i == 0), stop=(i == NCHUNK_W - 1),
                            )
                            if i == NCHUNK_W - 1:
                                mm.then_inc(segd, 1)
                    acc_phase(it, k)

        # ---------------- DVE ----------------
        @block.vector
        def _(dv: bass.BassVectorEngine):
            from concourse import mybir as mb
            dv.wait_ge(io, 16 * NPRO)
            # build W in SBUF: W[st, ch, :] = (iota == rr[st,ch]) * val[st,ch]
            for ch in range(NCHUNK):
                dv.tensor_scalar(
                    out=w_all[:, ch, :], in0=iota_sb[:],
                    scalar1=wr_sb[:, ch:ch + 1], scalar2=wv_sb[:, ch:ch + 1],
                    op0=mb.AluOpType.is_equal, op1=mb.AluOpType.mult,
                )
            dv.engine_nop().then_inc(wbd, 1)

            def acc_dve(it, ka):
                if no_acc:
                    return
                for s in range(NWIN):
                    ST = it * STRIPS + ka * NWIN + s
                    dv.wait_ge(accmm, ST + 1)
                    dv.tensor_tensor(
                        out=acc_strip(s), in0=acc_strip(s), in1=psum_acc[ST % 2][:, :F256],
                        op=mb.AluOpType.add,
                    ).then_inc(accfree, 1)

            for it in range(repeat):
                for b in range(NWIN // 2):
                    dv.tensor_copy(out=acc[:, b, :], in_=bias_sb[:])
                acc_dve(it, 0)
                for k in range(1, RANK):
                    if cheb_act:
                        acc_dve(it, k)
                        continue
                    S = it * 4 + k - 1
                    for p in range(NWIN):
                        P = S * NWIN + p
                        dv.wait_ge(segd, P + 1)
                        if cheb_sbuf:
                            op = dv.scalar_tensor_tensor(
                                out=ql_strip(k, p), in0=obf[0:64, 0, :], scalar=2.0,
                                in1=ql_strip(k - 2, p) if k >= 2 else ql_strip(0, p),
                                op0=mb.AluOpType.mult, op1=mb.AluOpType.subtract,
                            )
                        elif no_cheb:
                            op = dv.engine_nop()
                        elif k == 1:
                            op = dv.tensor_copy(out=ql_strip(1, p), in_=psum_seg[P % 2][:, :F256])
                        else:
                            op = dv.scalar_tensor_tensor(
                                out=ql_strip(k, p), in0=psum_seg[P % 2][:, :F256], scalar=2.0,
                                in1=ql_strip(k - 2, p),
                                op0=mb.AluOpType.mult, op1=mb.AluOpType.subtract,
                            )
                        op.then_inc(chebd, 1)
                        dv.engine_nop().then_inc(psfree, 1)
                    acc_dve(it, k)
                if no_acc:
                    dv.wait_ge(chebd, (it + 1) * 4 * NWIN)
                    dv.engine_nop().then_inc(outs, 1)
                else:
                    dv.wait_ge(accfree, (it + 1) * STRIPS)
                    dv.tensor_copy(out=obf[:], in_=acc[:]).then_inc(outs, 1)

        # ---------------- ACT: psum_tp -> fm drains (+ cheb_act ablation) ----------------
        @block.scalar
        def _(ac: bass.BassScalarEngine):
            ac.wait_ge(io, 16 * NPRO)
            if not no_acc:
                for it in range(repeat):
                    for ka in range(RANK):
                        for s in range(NWIN):
                            ST = it * STRIPS + ka * NWIN + s
                            if ST >= 2:
                                ac.wait_ge(accmm, ST - 1)  # fm slot free (PE mms of ST-2 done)
                            for h in range(2):
                                t = 2 * ST + h
                                ac.wait_ge(tpd, t + 1)
                                ac.activation(
                                    out=fm[:, ST % 2, h, :], in_=psum_tp[t % 2][:, :WROWS],
                                    func=mybir.ActivationFunctionType.Copy,
                                ).then_inc(fmcp, 1)
        if cheb_act:
            @block.scalar
            def _(ac: bass.BassScalarEngine):
                for it in range(repeat):
                    for k in range(1, RANK):
                        S = it * 4 + k - 1
                        for p in range(NWIN):
                            P = S * NWIN + p
                            ac.wait_ge(segd, P + 1)
                            ac.activation(
                                out=ql_strip(k, p), in_=psum_seg[P % 2][:, :F256],
                                func=mybir.ActivationFunctionType.Copy, scale=2.0,
                            ).then_inc(chebd, 1)

        # ---------------- SYNC: prologue loads, writeback, xs reload, output ----------------
        @block.sync
        def _(sy: bass.BassEngine):
            sy.dma_start(out=ql[:, 0, :, :], in_=xs_d[:]).then_inc(io, 16)
            for r in range(8):
                sy.dma_start(
                    out=idx_sb[16 * r:16 * r + 16, :], in_=idx_d[:],
                ).then_inc(io, 16)
            sy.dma_start(out=wv_sb[:], in_=wv_d[:]).then_inc(io, 16)
            sy.dma_start(out=wr_sb[:], in_=wr_d[:]).then_inc(io, 16)
            sy.dma_start(out=krep_sb[:], in_=krep_d[:]).then_inc(io, 16)
            sy.dma_start(out=bias_sb[:], in_=bias_d[:]).then_inc(io, 16)
            sy.dma_start(out=id_sb[:], in_=id_d[:]).then_inc(io, 16)
            sy.dma_start(out=iota_sb[:], in_=iota_d[:]).then_inc(io, 16)
            sy.wait_ge(io, 16 * NPRO)
            wbv0 = wb_l[0][:].rearrange(
                "(w2 two p) f -> (two p) w2 f", two=2, p=64)
            sy.dma_start(out=wbv0, in_=ql[:, 0, :, :]).then_inc(wbs[0], 16)
            for it in range(repeat):
                if it >= 1:
                    # reload xs into ql gen0 (overwritten by T_2/T_4 of prev iter);
                    # wait for all prev-iter transposes (last readers of gen0)
                    if no_acc:
                        sy.wait_ge(chebd, 4 * NWIN * it)
                    else:
                        sy.wait_ge(tpd, 2 * STRIPS * it)
                    sy.dma_start(out=ql[:, 0, :, :], in_=xs_d[:]).then_inc(xsl, 16)
                for k in range(1, 4):
                    sy.wait_ge(chebd, it * 4 * NWIN + k * NWIN)
                    wbv = wb_l[k][:].rearrange(
                        "(w2 two p) f -> (two p) w2 f", two=2, p=64)
                    sy.dma_start(out=wbv, in_=ql[:, k % 2, :, :]).then_inc(wbs[k], 16)
                sy.wait_ge(outs, it * 17 + 1)
                sy.dma_start(out=out_d[:], in_=obf[:]).then_inc(outs, 16)
            sy.wait_ge(outs, repeat * 17)

    nc.compile()
    return nc


def _make_in_maps(x, vals, kern, bias, rows, cols):
    import hashlib
    hk = ("host", hashlib.sha1(vals.tobytes()).hexdigest(),
          hashlib.sha1(rows.tobytes()).hexdigest(),
          hashlib.sha1(cols.tobytes()).hexdigest(),
          hashlib.sha1(kern.tobytes()).hexdigest(),
          hashlib.sha1(bias.tobytes()).hexdigest())
    if hk not in _cache:
        _cache[hk] = _build_host_data(vals, kern, bias, rows, cols)
    idx_all, wv_all, wr_all, krep, bias_rep, ident, iota = _cache[hk]

    x_table = x.transpose(1, 0, 2).reshape(M, F256).astype(BF16)  # [m, 32n+f]
    in_maps = []
    for c in range(NCORES):
        in_maps.append({
            "xs": _strip_layout(x_table, c),
            "idx": idx_all[c],
            "wv": wv_all[c],
            "wr": wr_all[c],
            "krep": krep,
            "biasr": bias_rep,
            "ident": ident,
            "iotar": iota,
        })
    return in_maps


def _postprocess(res):
    # unshard: per-core strips [128, 32, 256] -> rows [4096, 256]
    parts = []
    for c in range(NCORES):
        o = np.asarray(res.results[c]["out"]).reshape(2, WROWS, NWIN // 2, F256)
        parts.append(o.transpose(2, 0, 1, 3).reshape(QROWS, F256))
    full = np.concatenate(parts, axis=0).astype(np.float32)      # [M, 256]
    return np.ascontiguousarray(
        full.reshape(M, NB, FILT).transpose(1, 0, 2))            # [NB, M, FILT]


def kernel(x, vals, kernel, bias, rows, cols):
    from concourse.bass_utils import run_bass_kernel_spmd

    x = np.asarray(x, dtype=np.float32)
    vals = np.asarray(vals, dtype=np.float32)
    kern = np.asarray(kernel, dtype=np.float32)
    bias = np.asarray(bias, dtype=np.float32)
    rows = np.asarray(rows, dtype=np.int64)
    cols = np.asarray(cols, dtype=np.int64)

    if "nc" not in _cache:
        _cache["nc"] = _build_nc()
    nc = _cache["nc"]
    in_maps = _make_in_maps(x, vals, kern, bias, rows, cols)
    res = run_bass_kernel_spmd(nc, in_maps, core_ids=list(range(NCORES)))
    return _postprocess(res)
